# revision 9
# baseline (speedup 1.0000x reference)
"""Trainium2 Bass kernel for nn_MultiHeadAttention (B=4, S=2048, D=1024, H=16, causal, RoPE).

Sharding: 8 cores = 4 batches x 2 head-groups (8 heads each).
Each core computes q/k/v projections for its 512 head-dims, causal attention
for its 8 heads over its batch, and a partial o_proj; the host sums the two
partial o_proj outputs per batch (tensor-parallel reduce done host-side).

Device layouts are transposed ([dims, S]) so that:
  - scores are computed as sT[keys, queries] via K=32 row-packed matmuls
    (4 heads concurrently in the 128x128 PE array),
  - the PV matmul uses v[keys, dims] as the stationary operand with a ones
    column appended, so the softmax denominator falls out of the same matmul,
  - the o_proj matmul consumes the attention output without any transposes.

RoPE: host permutes q/k projection columns into de-interleaved (evens|odds)
blocks; interleaved rope then becomes 6 full-width DVE ops per tile pair, and
scores are invariant to the per-head permutation so nothing is permuted back.
Softmax skips max-subtraction (scores ~ N(0,1); no overflow) and applies the
causal mask as a single shifted-triangle multiply on diagonal blocks only.
"""

import contextlib
import ctypes
import sys
import types

sys.path.insert(0, "/opt/trn_rl_repo")

import numpy as np

import concourse.bass as bass
import concourse.tile as tile
from concourse import bass_utils, mybir
from concourse.vector_clock import ScopedClock

B, S, D = 4, 2048, 1024
H = 16
DK = 64
HG = 2              # head groups (cores per batch)
HL = H // HG        # heads per core = 8
DH = HL * DK        # head dims per core = 512
THETA = 10000.0
N_CORES = 8

F32 = mybir.dt.float32
BF16 = mybir.dt.bfloat16

_PATCHED = False
_NC_CACHE = {}


def _install_patches():
    """Environment fixes: split drain waits (this walrus rejects >2 waits per
    instruction), skip remote artifact upload, install the NTFF profile hook."""
    global _PATCHED
    if _PATCHED:
        return
    _PATCHED = True

    def patched_drain_and_barrier(self, tick_clock, wait_clock):
        nc = self.nc
        scratch = mybir.InstDrain(name="drain-wait-scratch", ins=[], outs=[])
        scratch.sync_info = mybir.SyncInfo(on_wait=[], on_update=[])
        scratch.engine = mybir.EngineType.SP
        wait_clock.add_sem_waits(scratch, ScopedClock({None: tick_clock.global_clock}))
        by_name = {s.name: s for s in self.sems.allocated().values()}
        for ent in scratch.sync_info.on_wait:
            nc.sync.wait_ge(by_name[ent.ant_name], ent.wait_value)
        nc.sync.drain()
        nc.all_engine_barrier()
        popped = nc._tile_sem_poison_stack.pop()
        assert popped is self._sem_poison
        nc.clear_and_free_semaphores(list(self.sems.allocated().values()))
        nc.all_engine_barrier()

    tile.TileContext._drain_and_barrier = patched_drain_and_barrier

    # this walrus accepts at most ONE sync wait per instruction: hoist excess
    # waits onto same-engine InstNoOp carriers just before the instruction.
    # Safe because Tile waits only ever point backward in the schedule order.
    orig_cal = tile.TileContext._commit_and_lower
    ws_counter = [0]

    def patched_commit_and_lower(self, inst, original_block, old_bb_map,
                                 bb_to_exit_bb):
        si = getattr(inst, "sync_info", None)
        if si is not None and si.on_wait and len(si.on_wait) > 1:
            waits = list(si.on_wait)
            for w in waits[:-1]:
                ws_counter[0] += 1
                nop = mybir.InstNoOp(
                    name=f"waitsplit-{ws_counter[0]}",
                    sync_info=mybir.SyncInfo(on_wait=[w], on_update=[]),
                    bass_nofuse=True,
                    engine=inst.engine,
                )
                self._commit_instruction(nop, lazy_reg_writes=False)
            inst.sync_info = mybir.SyncInfo(
                on_wait=[waits[-1]], on_update=list(si.on_update))
        return orig_cal(self, inst, original_block, old_bb_map, bb_to_exit_bb)

    tile.TileContext._commit_and_lower = patched_commit_and_lower
    bass_utils.upload_artifacts = lambda tmpdir: str(tmpdir)

    so_path = "/opt/axon/libaxon_pjrt.so"
    hook = None
    try:
        lib = ctypes.CDLL(so_path)
        if hasattr(lib, "axon_start_nrt_profile"):
            lib.axon_start_nrt_profile.argtypes = [
                ctypes.POINTER(ctypes.c_int64), ctypes.c_size_t]
            lib.axon_start_nrt_profile.restype = ctypes.c_int64
            lib.axon_stop_nrt_profile.argtypes = [ctypes.c_char_p]
            lib.axon_stop_nrt_profile.restype = ctypes.c_int64

            @contextlib.contextmanager
            def _hook(output_dir, device_ids):
                import jax
                jax.devices()
                if device_ids:
                    ids = (ctypes.c_int64 * len(device_ids))(*device_ids)
                    rc = lib.axon_start_nrt_profile(ids, len(device_ids))
                else:
                    rc = lib.axon_start_nrt_profile(None, 0)
                if rc != 0:
                    raise RuntimeError(f"axon_start_nrt_profile rc={rc}")
                try:
                    yield
                finally:
                    n = lib.axon_stop_nrt_profile(str(output_dir).encode())
                    print(f"ntff profile: {n} file(s) -> {output_dir}")

            hook = _hook
    except OSError:
        pass

    import antenv
    mod = types.ModuleType("antenv.axon_hooks")
    mod.get_axon_ntff_profile_hook = lambda: hook
    mod.set_axon_ntff_profile_hook = lambda h: None
    sys.modules["antenv.axon_hooks"] = mod
    antenv.axon_hooks = mod


def build_nc(seq=S):
    """One SPMD program; per-core differences are input data only."""
    QC = 512                      # query-chunk width (= one PSUM bank of f32)
    n_qc = seq // QC              # query chunks
    n_kt = seq // 128             # key tiles
    n_st = seq // 128             # s tiles (o_proj M)
    KT = 8                        # contraction tiles for projections (D/128)

    nc = bass.Bass(target_bir_lowering=False)

    xT_d = nc.dram_tensor("xT", [D, seq], F32, kind="ExternalInput")
    wq_d = nc.dram_tensor("wq", [D, DH], F32, kind="ExternalInput")
    wk_d = nc.dram_tensor("wk", [D, DH], F32, kind="ExternalInput")
    wv_d = nc.dram_tensor("wv", [D, DH], F32, kind="ExternalInput")
    wo_d = nc.dram_tensor("wo", [DH, D], F32, kind="ExternalInput")
    cosr_d = nc.dram_tensor("cosr", [128, seq], F32, kind="ExternalInput")
    sinr_d = nc.dram_tensor("sinr", [128, seq], F32, kind="ExternalInput")
    wm_d = nc.dram_tensor("wmask", [128, 896], F32, kind="ExternalInput")
    out_d = nc.dram_tensor("out", [seq, D], F32, kind="ExternalOutput")

    with tile.TileContext(nc) as tc:
        with contextlib.ExitStack() as ctx:
            res = ctx.enter_context(tc.tile_pool(name="res", bufs=1))
            stage = ctx.enter_context(tc.tile_pool(name="stage", bufs=2))
            ropet = ctx.enter_context(tc.tile_pool(name="ropet", bufs=4))
            pts = ctx.enter_context(tc.tile_pool(name="pts", bufs=6))
            nrm = ctx.enter_context(tc.tile_pool(name="nrm", bufs=4))
            dsc = ctx.enter_context(
                tc.tile_pool(name="dsc", bufs=4, space="DRAM"))
            psum = ctx.enter_context(
                tc.tile_pool(name="psum", bufs=8, space="PSUM"))
            ps_proj = ps_sc = ps_pv = ps_o = psum

            # ---- load + cast ------------------------------------------------
            def load_cast(dram, rows, cols, n_tiles, name):
                tiles = []
                for k in range(n_tiles):
                    st = stage.tile([128, cols], F32, tag="stage")
                    nc.sync.dma_start(st[:], dram[k * 128:(k + 1) * 128, :])
                    bt = res.tile([128, cols], BF16, name=f"{name}{k}", tag=f"{name}{k}")
                    nc.scalar.copy(bt[:], st[:])
                    tiles.append(bt)
                return tiles

            xT = load_cast(xT_d, D, seq, 8, "xT")
            wq = load_cast(wq_d, D, DH, 8, "wq")
            wk = load_cast(wk_d, D, DH, 8, "wk")
            wv = load_cast(wv_d, D, DH, 8, "wv")
            wo = load_cast(wo_d, DH, D, 4, "wo")

            cosr = res.tile([128, seq], F32, name="cosr", tag="cosr")
            nc.sync.dma_start(cosr[:], cosr_d[:])
            sinr = res.tile([128, seq], F32, name="sinr", tag="sinr")
            nc.sync.dma_start(sinr[:], sinr_d[:])
            wm = res.tile([128, 896], F32, name="wm", tag="wm")
            nc.sync.dma_start(wm[:], wm_d[:])

            # ---- projections -----------------------------------------------
            # q/k: psum[m][qc] for M-tile pairs (lo tile 2j, hi tile 2j+1),
            # rope applied on the pair -> qT/kT bf16 [128, seq] x4.
            qT = [res.tile([128, seq], BF16, name=f"qT{m}", tag=f"qT{m}")
                  for m in range(4)]
            kTt = [res.tile([128, seq], BF16, name=f"kT{m}", tag=f"kT{m}") for m in range(4)]

            def project_rope(w_tiles, dst):
                for j in range(2):           # 4-head block (tile pair)
                    ps_pair = []
                    for m in (2 * j, 2 * j + 1):
                        ps_m = []
                        for qc in range(n_qc):
                            pst = ps_proj.tile([128, QC], F32, tag="ps", name="pst")
                            for k in range(KT):
                                nc.tensor.matmul(
                                    pst[:],
                                    w_tiles[k][:, m * 128:(m + 1) * 128],
                                    xT[k][:, qc * QC:(qc + 1) * QC],
                                    start=(k == 0), stop=(k == KT - 1))
                            ps_m.append(pst)
                        ps_pair.append(ps_m)
                    lo_ps, hi_ps = ps_pair
                    for qc in range(n_qc):
                        cs = cosr[:, qc * QC:(qc + 1) * QC]
                        sn = sinr[:, qc * QC:(qc + 1) * QC]
                        ta = ropet.tile([128, QC], F32, tag="ropet")
                        tb = ropet.tile([128, QC], F32, tag="ropet")
                        nc.vector.tensor_mul(ta[:], lo_ps[qc][:], cs)
                        nc.vector.tensor_mul(tb[:], hi_ps[qc][:], sn)
                        nc.vector.tensor_sub(
                            dst[2 * j][:, qc * QC:(qc + 1) * QC], ta[:], tb[:])
                        tc2 = ropet.tile([128, QC], F32, tag="ropet")
                        td = ropet.tile([128, QC], F32, tag="ropet")
                        nc.vector.tensor_mul(tc2[:], hi_ps[qc][:], cs)
                        nc.vector.tensor_mul(td[:], lo_ps[qc][:], sn)
                        nc.vector.tensor_add(
                            dst[2 * j + 1][:, qc * QC:(qc + 1) * QC],
                            tc2[:], td[:])

            project_rope(wq, qT)
            project_rope(wk, kTt)

            # v: natural [s, dims] layout, 65-wide per head with a ones column.
            v_sb = []
            for st_i in range(n_st):
                pst = ps_proj.tile([128, DH], F32, tag="ps", name="pst")
                for k in range(KT):
                    nc.tensor.matmul(
                        pst[:],
                        xT[k][:, st_i * 128:(st_i + 1) * 128],
                        wv[k][:],
                        start=(k == 0), stop=(k == KT - 1))
                vt = res.tile([128, HL, DK + 1], BF16, name=f"v{st_i}", tag=f"v{st_i}")
                nc.scalar.copy(
                    vt[:, :, 0:DK],
                    pst[:].rearrange("p (h d) -> p h d", h=HL))
                nc.vector.memset(vt[:, :, DK:DK + 1], 1.0)
                v_sb.append(vt)

            # ---- attention --------------------------------------------------
            aoT = [res.tile([128, seq], BF16, name=f"aoT{t}", tag=f"aoT{t}") for t in range(4)]
            SCALE = 1.0 / np.sqrt(np.float32(DK))

            for g2 in range(4):               # pairs of heads
                j = g2 // 2                   # which qT/kT tile pair
                lo_t, hi_t = qT[2 * j], qT[2 * j + 1]
                klo_t, khi_t = kTt[2 * j], kTt[2 * j + 1]
                for qc in range(n_qc):
                    pv_ps = [ps_pv.tile([DK + 1, QC], F32, tag="ps", name="pv_ps")
                             for _ in range(2)]
                    kt_hi = min(n_kt, 4 * (qc + 1))
                    for kt in range(kt_hi):
                        sc_ps = [ps_sc.tile([128, QC], F32, tag="ps", name="sc_ps")
                                 for _ in range(2)]
                        for i in range(2):
                            h4 = (g2 % 2) * 2 + i      # row group within pair
                            rb = 32 * h4
                            tp = (rb, 0)
                            nc.tensor.matmul(
                                sc_ps[i][:],
                                klo_t[rb:rb + 32, kt * 128:(kt + 1) * 128],
                                lo_t[rb:rb + 32, qc * QC:(qc + 1) * QC],
                                start=True, stop=False, tile_position=tp)
                            nc.tensor.matmul(
                                sc_ps[i][:],
                                khi_t[rb:rb + 32, kt * 128:(kt + 1) * 128],
                                hi_t[rb:rb + 32, qc * QC:(qc + 1) * QC],
                                start=False, stop=True, tile_position=tp)
                        for i in range(2):
                            h = 2 * g2 + i
                            pt = pts.tile([128, QC], BF16, tag="pts")
                            nc.scalar.activation(
                                pt[:], sc_ps[i][:],
                                mybir.ActivationFunctionType.Exp, scale=SCALE)
                            r = kt - 4 * qc
                            if r >= 0:
                                off = 384 - 128 * r
                                nc.vector.tensor_mul(
                                    pt[:], pt[:], wm[:, off:off + QC])
                            nc.tensor.matmul(
                                pv_ps[i][:],
                                v_sb[kt][:, h, :],
                                pt[:],
                                start=(kt == 0), stop=(kt == kt_hi - 1))
                    for i in range(2):
                        h = 2 * g2 + i
                        rt = nrm.tile([1, QC], F32, tag="rt")
                        nc.vector.reciprocal(
                            rt[:, :], pv_ps[i][DK:DK + 1, :])
                        dt_s = dsc.tile([1, QC], F32, tag="dsc", name="dt_s")
                        nc.sync.dma_start(dt_s[:], rt[:])
                        rbc = nrm.tile([64, QC], F32, tag="rbc")
                        bcast = bass.AP(
                            dt_s.tensor, dt_s.offset,
                            [[0, 64]] + [list(a) for a in dt_s[:].ap[1:]])
                        nc.sync.dma_start(rbc[:], bcast)
                        nc.vector.tensor_mul(
                            aoT[h // 2][(h % 2) * 64:(h % 2) * 64 + 64,
                                        qc * QC:(qc + 1) * QC],
                            pv_ps[i][0:DK, :], rbc[:])

            # ---- o_proj -----------------------------------------------------
            for st_i in range(n_st):
                for oc in range(2):
                    pso = ps_o.tile([128, 512], F32, tag="ps", name="pso")
                    for k4 in range(4):
                        nc.tensor.matmul(
                            pso[:],
                            aoT[k4][:, st_i * 128:(st_i + 1) * 128],
                            wo[k4][:, oc * 512:(oc + 1) * 512],
                            start=(k4 == 0), stop=(k4 == 3))
                    ot = pts.tile([128, 512], F32, tag="ot", name="ot")
                    nc.scalar.copy(ot[:], pso[:])
                    nc.sync.dma_start(
                        out_d[st_i * 128:(st_i + 1) * 128,
                              oc * 512:(oc + 1) * 512],
                        ot[:])
    return nc


def prepare_inputs(x, q_proj, k_proj, v_proj, o_proj, token_positions, seq=S):
    """Shard + lay out host-side. Returns one in_map per core."""
    x = np.asarray(x, dtype=np.float32)
    q_proj = np.asarray(q_proj, dtype=np.float32)
    k_proj = np.asarray(k_proj, dtype=np.float32)
    v_proj = np.asarray(v_proj, dtype=np.float32)
    o_proj = np.asarray(o_proj, dtype=np.float32)
    pos = np.asarray(token_positions)

    # rope tables (exactly mirrors reference._rope_tables + gather)
    dims = np.arange(0, DK, 2, dtype=np.float32)
    freqs = 1.0 / THETA ** (dims / DK)
    t = np.arange(2048, dtype=np.float32)
    angles = np.outer(t, freqs)                      # (2048, 32)
    cos_tab = np.cos(angles)[pos].astype(np.float32)  # (seq, 32)
    sin_tab = np.sin(angles)[pos].astype(np.float32)
    cosr = np.tile(np.ascontiguousarray(cos_tab.T), (4, 1))  # (128, seq)
    sinr = np.tile(np.ascontiguousarray(sin_tab.T), (4, 1))

    # shifted causal mask: wm[k, c] = 1 iff c >= k + 384
    kk = np.arange(128)[:, None]
    cc = np.arange(896)[None, :]
    wm = (cc >= kk + 384).astype(np.float32)

    in_maps = []
    for c in range(N_CORES):
        b, hg = c // 2, c % 2
        # column permutation for q/k: per 4-head block, evens of 4 heads
        # (lo tile) then odds of 4 heads (hi tile)
        cols = []
        for j in range(2):
            for par in range(2):            # 0: evens (lo), 1: odds (hi)
                for h4 in range(4):
                    head = hg * HL + 4 * j + h4
                    cols.extend(64 * head + 2 * np.arange(32) + par)
        cols = np.asarray(cols)
        hslice = slice(hg * DH, (hg + 1) * DH)
        in_maps.append({
            "xT": np.ascontiguousarray(x[b, :seq, :].T),
            "wq": np.ascontiguousarray(q_proj[:, cols]),
            "wk": np.ascontiguousarray(k_proj[:, cols]),
            "wv": np.ascontiguousarray(v_proj[:, hslice]),
            "wo": np.ascontiguousarray(o_proj[hslice, :]),
            "cosr": cosr[:, :seq].copy(),
            "sinr": sinr[:, :seq].copy(),
            "wmask": wm,
        })
    return in_maps


def run(inputs, seq=S, trace=False, tmpdir=None):
    _install_patches()
    if seq not in _NC_CACHE:
        _NC_CACHE[seq] = build_nc(seq)
    nc = _NC_CACHE[seq]
    in_maps = prepare_inputs(**inputs, seq=seq)
    kw = {}
    if trace:
        kw = dict(trace=True, tmpdir=tmpdir)
    res = bass_utils.run_bass_kernel_spmd(
        nc, in_maps, core_ids=list(range(N_CORES)), **kw)
    parts = [res.results[c]["out"] for c in range(N_CORES)]
    out = np.stack([parts[2 * b] + parts[2 * b + 1] for b in range(B)])
    return out, res


def kernel(x, q_proj, k_proj, v_proj, o_proj, token_positions):
    out, _ = run(dict(x=x, q_proj=q_proj, k_proj=k_proj, v_proj=v_proj,
                      o_proj=o_proj, token_positions=token_positions))
    return out


# revision 10
# speedup vs baseline: 1.1364x; 1.1364x over previous
"""Trainium2 Bass kernel for nn_MultiHeadAttention (B=4, S=2048, D=1024, H=16, causal, RoPE).

Sharding: 8 cores = 4 batches x 2 head-groups (8 heads each).
Each core computes q/k/v projections for its 512 head-dims, causal attention
for its 8 heads over its batch, and a partial o_proj; the host sums the two
partial o_proj outputs per batch (tensor-parallel reduce done host-side).

Device layouts are transposed ([dims, S]) so that:
  - scores are computed as sT[keys, queries] via K=32 row-packed matmuls
    (4 heads concurrently in the 128x128 PE array),
  - the PV matmul uses v[keys, dims] as the stationary operand with a ones
    column appended, so the softmax denominator falls out of the same matmul,
  - the o_proj matmul consumes the attention output without any transposes.

RoPE: host permutes q/k projection columns into de-interleaved (evens|odds)
blocks; interleaved rope then becomes 6 full-width DVE ops per tile pair, and
scores are invariant to the per-head permutation so nothing is permuted back.
Softmax skips max-subtraction (scores ~ N(0,1); no overflow) and applies the
causal mask as a single shifted-triangle multiply on diagonal blocks only.
"""

import contextlib
import ctypes
import sys
import types

sys.path.insert(0, "/opt/trn_rl_repo")

import numpy as np

import concourse.bass as bass
import concourse.tile as tile
from concourse import bass_utils, mybir
from concourse.vector_clock import ScopedClock

B, S, D = 4, 2048, 1024
H = 16
DK = 64
HG = 2              # head groups (cores per batch)
HL = H // HG        # heads per core = 8
DH = HL * DK        # head dims per core = 512
THETA = 10000.0
N_CORES = 8

F32 = mybir.dt.float32
BF16 = mybir.dt.bfloat16

_PATCHED = False
_NC_CACHE = {}


def _install_patches():
    """Environment fixes: split drain waits (this walrus rejects >2 waits per
    instruction), skip remote artifact upload, install the NTFF profile hook."""
    global _PATCHED
    if _PATCHED:
        return
    _PATCHED = True

    def patched_drain_and_barrier(self, tick_clock, wait_clock):
        nc = self.nc
        scratch = mybir.InstDrain(name="drain-wait-scratch", ins=[], outs=[])
        scratch.sync_info = mybir.SyncInfo(on_wait=[], on_update=[])
        scratch.engine = mybir.EngineType.SP
        wait_clock.add_sem_waits(scratch, ScopedClock({None: tick_clock.global_clock}))
        by_name = {s.name: s for s in self.sems.allocated().values()}
        for ent in scratch.sync_info.on_wait:
            nc.sync.wait_ge(by_name[ent.ant_name], ent.wait_value)
        nc.sync.drain()
        nc.all_engine_barrier()
        popped = nc._tile_sem_poison_stack.pop()
        assert popped is self._sem_poison
        nc.clear_and_free_semaphores(list(self.sems.allocated().values()))
        nc.all_engine_barrier()

    tile.TileContext._drain_and_barrier = patched_drain_and_barrier

    # this walrus accepts at most ONE sync wait per instruction: hoist excess
    # waits onto same-engine InstNoOp carriers just before the instruction.
    # Safe because Tile waits only ever point backward in the schedule order.
    orig_cal = tile.TileContext._commit_and_lower
    ws_counter = [0]

    def patched_commit_and_lower(self, inst, original_block, old_bb_map,
                                 bb_to_exit_bb):
        si = getattr(inst, "sync_info", None)
        if si is not None and si.on_wait and len(si.on_wait) > 1:
            waits = list(si.on_wait)
            for w in waits[:-1]:
                ws_counter[0] += 1
                nop = mybir.InstNoOp(
                    name=f"waitsplit-{ws_counter[0]}",
                    sync_info=mybir.SyncInfo(on_wait=[w], on_update=[]),
                    bass_nofuse=True,
                    engine=inst.engine,
                )
                self._commit_instruction(nop, lazy_reg_writes=False)
            inst.sync_info = mybir.SyncInfo(
                on_wait=[waits[-1]], on_update=list(si.on_update))
        return orig_cal(self, inst, original_block, old_bb_map, bb_to_exit_bb)

    tile.TileContext._commit_and_lower = patched_commit_and_lower
    bass_utils.upload_artifacts = lambda tmpdir: str(tmpdir)

    so_path = "/opt/axon/libaxon_pjrt.so"
    hook = None
    try:
        lib = ctypes.CDLL(so_path)
        if hasattr(lib, "axon_start_nrt_profile"):
            lib.axon_start_nrt_profile.argtypes = [
                ctypes.POINTER(ctypes.c_int64), ctypes.c_size_t]
            lib.axon_start_nrt_profile.restype = ctypes.c_int64
            lib.axon_stop_nrt_profile.argtypes = [ctypes.c_char_p]
            lib.axon_stop_nrt_profile.restype = ctypes.c_int64

            @contextlib.contextmanager
            def _hook(output_dir, device_ids):
                import jax
                jax.devices()
                if device_ids:
                    ids = (ctypes.c_int64 * len(device_ids))(*device_ids)
                    rc = lib.axon_start_nrt_profile(ids, len(device_ids))
                else:
                    rc = lib.axon_start_nrt_profile(None, 0)
                if rc != 0:
                    raise RuntimeError(f"axon_start_nrt_profile rc={rc}")
                try:
                    yield
                finally:
                    n = lib.axon_stop_nrt_profile(str(output_dir).encode())
                    print(f"ntff profile: {n} file(s) -> {output_dir}")

            hook = _hook
    except OSError:
        pass

    import antenv
    mod = types.ModuleType("antenv.axon_hooks")
    mod.get_axon_ntff_profile_hook = lambda: hook
    mod.set_axon_ntff_profile_hook = lambda h: None
    sys.modules["antenv.axon_hooks"] = mod
    antenv.axon_hooks = mod


def build_nc(seq=S):
    """One SPMD program; per-core differences are input data only."""
    QC = 512                      # query-chunk width (= one PSUM bank of f32)
    n_qc = seq // QC              # query chunks
    n_kt = seq // 128             # key tiles
    n_st = seq // 128             # s tiles (o_proj M)
    KT = 8                        # contraction tiles for projections (D/128)

    nc = bass.Bass(target_bir_lowering=False)

    xT_d = nc.dram_tensor("xT", [D, seq], F32, kind="ExternalInput")
    wq_d = nc.dram_tensor("wq", [D, DH], F32, kind="ExternalInput")
    wk_d = nc.dram_tensor("wk", [D, DH], F32, kind="ExternalInput")
    wv_d = nc.dram_tensor("wv", [D, DH], F32, kind="ExternalInput")
    wo_d = nc.dram_tensor("wo", [DH, D], F32, kind="ExternalInput")
    cosr_d = nc.dram_tensor("cosr", [128, seq], F32, kind="ExternalInput")
    sinr_d = nc.dram_tensor("sinr", [128, seq], F32, kind="ExternalInput")
    wm_d = nc.dram_tensor("wmask", [128, 896], F32, kind="ExternalInput")
    out_d = nc.dram_tensor("out", [seq, D], F32, kind="ExternalOutput")

    with tile.TileContext(nc) as tc:
        with contextlib.ExitStack() as ctx:
            res = ctx.enter_context(tc.tile_pool(name="res", bufs=1))
            stage = ctx.enter_context(tc.tile_pool(name="stage", bufs=2))
            ropet = ctx.enter_context(tc.tile_pool(name="ropet", bufs=4))
            pts = ctx.enter_context(tc.tile_pool(name="pts", bufs=6))
            nrm = ctx.enter_context(tc.tile_pool(name="nrm", bufs=4))
            dsc = ctx.enter_context(
                tc.tile_pool(name="dsc", bufs=4, space="DRAM"))
            psum = ctx.enter_context(
                tc.tile_pool(name="psum", bufs=8, space="PSUM"))
            ps_proj = ps_sc = ps_pv = ps_o = psum

            # ---- load + cast ------------------------------------------------
            def load_cast(dram, rows, cols, n_tiles, name):
                tiles = []
                for k in range(n_tiles):
                    st = stage.tile([128, cols], F32, tag="stage")
                    nc.sync.dma_start(st[:], dram[k * 128:(k + 1) * 128, :])
                    bt = res.tile([128, cols], BF16, name=f"{name}{k}", tag=f"{name}{k}")
                    nc.scalar.copy(bt[:], st[:])
                    tiles.append(bt)
                return tiles

            xT = load_cast(xT_d, D, seq, 8, "xT")
            wq = load_cast(wq_d, D, DH, 8, "wq")
            wk = load_cast(wk_d, D, DH, 8, "wk")
            wv = load_cast(wv_d, D, DH, 8, "wv")
            wo = load_cast(wo_d, DH, D, 4, "wo")

            cosr = res.tile([128, seq], F32, name="cosr", tag="cosr")
            nc.sync.dma_start(cosr[:], cosr_d[:])
            sinr = res.tile([128, seq], F32, name="sinr", tag="sinr")
            nc.sync.dma_start(sinr[:], sinr_d[:])
            wm = res.tile([128, 896], F32, name="wm", tag="wm")
            nc.sync.dma_start(wm[:], wm_d[:])

            # ---- projections -----------------------------------------------
            # q/k: psum[m][qc] for M-tile pairs (lo tile 2j, hi tile 2j+1),
            # rope applied on the pair -> qT/kT bf16 [128, seq] x4.
            qT = [res.tile([128, seq], BF16, name=f"qT{m}", tag=f"qT{m}")
                  for m in range(4)]
            kTt = [res.tile([128, seq], BF16, name=f"kT{m}", tag=f"kT{m}") for m in range(4)]

            def project_rope(w_tiles, dst):
                for j in range(2):           # 4-head block (tile pair)
                    ps_pair = []
                    for m in (2 * j, 2 * j + 1):
                        ps_m = []
                        for qc in range(n_qc):
                            pst = ps_proj.tile([128, QC], F32, tag="ps", name="pst")
                            for k in range(KT):
                                nc.tensor.matmul(
                                    pst[:],
                                    w_tiles[k][:, m * 128:(m + 1) * 128],
                                    xT[k][:, qc * QC:(qc + 1) * QC],
                                    start=(k == 0), stop=(k == KT - 1))
                            ps_m.append(pst)
                        ps_pair.append(ps_m)
                    lo_ps, hi_ps = ps_pair
                    for qc in range(n_qc):
                        cs = cosr[:, qc * QC:(qc + 1) * QC]
                        sn = sinr[:, qc * QC:(qc + 1) * QC]
                        ta = ropet.tile([128, QC], F32, tag="ropet")
                        tb = ropet.tile([128, QC], F32, tag="ropet")
                        nc.vector.tensor_mul(ta[:], lo_ps[qc][:], cs)
                        nc.vector.tensor_mul(tb[:], hi_ps[qc][:], sn)
                        nc.vector.tensor_sub(
                            dst[2 * j][:, qc * QC:(qc + 1) * QC], ta[:], tb[:])
                        tc2 = ropet.tile([128, QC], F32, tag="ropet")
                        td = ropet.tile([128, QC], F32, tag="ropet")
                        nc.vector.tensor_mul(tc2[:], hi_ps[qc][:], cs)
                        nc.vector.tensor_mul(td[:], lo_ps[qc][:], sn)
                        nc.vector.tensor_add(
                            dst[2 * j + 1][:, qc * QC:(qc + 1) * QC],
                            tc2[:], td[:])

            project_rope(wq, qT)
            project_rope(wk, kTt)

            # v: natural [s, dims] layout, 65-wide per head with a ones column.
            v_sb = []
            for st_i in range(n_st):
                pst = ps_proj.tile([128, DH], F32, tag="ps", name="pst")
                for k in range(KT):
                    nc.tensor.matmul(
                        pst[:],
                        xT[k][:, st_i * 128:(st_i + 1) * 128],
                        wv[k][:],
                        start=(k == 0), stop=(k == KT - 1))
                vt = res.tile([128, HL, DK + 1], BF16, name=f"v{st_i}", tag=f"v{st_i}")
                nc.scalar.copy(
                    vt[:, :, 0:DK],
                    pst[:].rearrange("p (h d) -> p h d", h=HL))
                nc.vector.memset(vt[:, :, DK:DK + 1], 1.0)
                v_sb.append(vt)

            # ---- attention --------------------------------------------------
            aoT = [res.tile([128, seq], BF16, name=f"aoT{t}", tag=f"aoT{t}") for t in range(4)]
            SCALE = 1.0 / np.sqrt(np.float32(DK))

            # broadcast source for the softmax-recip: row 0 is live, rows
            # 1-31 are zero-initialized so stream_shuffle may stream them.
            rs = res.tile([32, QC], F32, name="rs", tag="rs")
            nc.vector.memset(rs[:], 0.0)

            def emit_norm(pv_pair, g2, qc):
                for i in range(2):
                    h = 2 * g2 + i
                    nc.vector.reciprocal(rs[0:1, :], pv_pair[i][DK:DK + 1, :])
                    rbc = nrm.tile([64, QC], F32, tag="rbc", name="rbc")
                    nc.vector.stream_shuffle(rbc[0:32, :], rs[:, :], [0] * 32)
                    nc.vector.stream_shuffle(rbc[32:64, :], rs[:, :], [0] * 32)
                    nc.vector.tensor_mul(
                        aoT[h // 2][(h % 2) * 64:(h % 2) * 64 + 64,
                                    qc * QC:(qc + 1) * QC],
                        pv_pair[i][0:DK, :], rbc[:])

            pending_norm = None   # deferred one qc so its latency hides
            for g2 in range(4):               # pairs of heads
                j = g2 // 2                   # which qT/kT tile pair
                lo_t, hi_t = qT[2 * j], qT[2 * j + 1]
                klo_t, khi_t = kTt[2 * j], kTt[2 * j + 1]
                for qc in range(n_qc):
                    pv_ps = [ps_pv.tile([DK + 1, QC], F32, tag="ps", name="pv_ps")
                             for _ in range(2)]
                    kt_hi = min(n_kt, 4 * (qc + 1))

                    def emit_sc(kt):
                        # diagonal tiles only need columns >= 128r
                        r = kt - 4 * qc
                        c0 = 128 * r if r > 0 else 0
                        sc_ps = [ps_sc.tile([128, QC], F32, tag="ps",
                                            name="sc_ps") for _ in range(2)]
                        for i in range(2):
                            h4 = (g2 % 2) * 2 + i   # row group within pair
                            rb = 32 * h4
                            tp = (rb, 0)
                            nc.tensor.matmul(
                                sc_ps[i][:, c0:QC],
                                klo_t[rb:rb + 32, kt * 128:(kt + 1) * 128],
                                lo_t[rb:rb + 32, qc * QC + c0:(qc + 1) * QC],
                                start=True, stop=False, tile_position=tp)
                            nc.tensor.matmul(
                                sc_ps[i][:, c0:QC],
                                khi_t[rb:rb + 32, kt * 128:(kt + 1) * 128],
                                hi_t[rb:rb + 32, qc * QC + c0:(qc + 1) * QC],
                                start=False, stop=True, tile_position=tp)
                        return kt, c0, sc_ps

                    def emit_px(kt, c0, sc_ps):
                        r = kt - 4 * qc
                        for i in range(2):
                            h = 2 * g2 + i
                            pt = pts.tile([128, QC], BF16, tag="pts", name="pt")
                            nc.scalar.activation(
                                pt[:, c0:QC], sc_ps[i][:, c0:QC],
                                mybir.ActivationFunctionType.Exp, scale=SCALE)
                            if r >= 0:
                                nc.vector.tensor_mul(
                                    pt[:, c0:c0 + 128], pt[:, c0:c0 + 128],
                                    wm[:, 384:512])
                            nc.tensor.matmul(
                                pv_ps[i][:, c0:QC],
                                v_sb[kt][:, h, :],
                                pt[:, c0:QC],
                                start=(kt == 0), stop=(kt == kt_hi - 1))

                    prev = None
                    for kt in range(kt_hi):
                        cur = emit_sc(kt)
                        if prev is not None:
                            emit_px(*prev)
                        prev = cur
                    emit_px(*prev)
                    if pending_norm is not None:
                        emit_norm(*pending_norm)
                    pending_norm = (pv_ps, g2, qc)
            emit_norm(*pending_norm)

            # ---- o_proj -----------------------------------------------------
            for st_i in range(n_st):
                for oc in range(2):
                    pso = ps_o.tile([128, 512], F32, tag="ps", name="pso")
                    for k4 in range(4):
                        nc.tensor.matmul(
                            pso[:],
                            aoT[k4][:, st_i * 128:(st_i + 1) * 128],
                            wo[k4][:, oc * 512:(oc + 1) * 512],
                            start=(k4 == 0), stop=(k4 == 3))
                    ot = pts.tile([128, 512], F32, tag="ot", name="ot")
                    nc.scalar.copy(ot[:], pso[:])
                    nc.sync.dma_start(
                        out_d[st_i * 128:(st_i + 1) * 128,
                              oc * 512:(oc + 1) * 512],
                        ot[:])
    return nc


def prepare_inputs(x, q_proj, k_proj, v_proj, o_proj, token_positions, seq=S):
    """Shard + lay out host-side. Returns one in_map per core."""
    x = np.asarray(x, dtype=np.float32)
    q_proj = np.asarray(q_proj, dtype=np.float32)
    k_proj = np.asarray(k_proj, dtype=np.float32)
    v_proj = np.asarray(v_proj, dtype=np.float32)
    o_proj = np.asarray(o_proj, dtype=np.float32)
    pos = np.asarray(token_positions)

    # rope tables (exactly mirrors reference._rope_tables + gather)
    dims = np.arange(0, DK, 2, dtype=np.float32)
    freqs = 1.0 / THETA ** (dims / DK)
    t = np.arange(2048, dtype=np.float32)
    angles = np.outer(t, freqs)                      # (2048, 32)
    cos_tab = np.cos(angles)[pos].astype(np.float32)  # (seq, 32)
    sin_tab = np.sin(angles)[pos].astype(np.float32)
    cosr = np.tile(np.ascontiguousarray(cos_tab.T), (4, 1))  # (128, seq)
    sinr = np.tile(np.ascontiguousarray(sin_tab.T), (4, 1))

    # shifted causal mask: wm[k, c] = 1 iff c >= k + 384
    kk = np.arange(128)[:, None]
    cc = np.arange(896)[None, :]
    wm = (cc >= kk + 384).astype(np.float32)

    in_maps = []
    for c in range(N_CORES):
        b, hg = c // 2, c % 2
        # column permutation for q/k: per 4-head block, evens of 4 heads
        # (lo tile) then odds of 4 heads (hi tile)
        cols = []
        for j in range(2):
            for par in range(2):            # 0: evens (lo), 1: odds (hi)
                for h4 in range(4):
                    head = hg * HL + 4 * j + h4
                    cols.extend(64 * head + 2 * np.arange(32) + par)
        cols = np.asarray(cols)
        hslice = slice(hg * DH, (hg + 1) * DH)
        in_maps.append({
            "xT": np.ascontiguousarray(x[b, :seq, :].T),
            "wq": np.ascontiguousarray(q_proj[:, cols]),
            "wk": np.ascontiguousarray(k_proj[:, cols]),
            "wv": np.ascontiguousarray(v_proj[:, hslice]),
            "wo": np.ascontiguousarray(o_proj[hslice, :]),
            "cosr": cosr[:, :seq].copy(),
            "sinr": sinr[:, :seq].copy(),
            "wmask": wm,
        })
    return in_maps


def run(inputs, seq=S, trace=False, tmpdir=None):
    _install_patches()
    if seq not in _NC_CACHE:
        _NC_CACHE[seq] = build_nc(seq)
    nc = _NC_CACHE[seq]
    in_maps = prepare_inputs(**inputs, seq=seq)
    kw = {}
    if trace:
        kw = dict(trace=True, tmpdir=tmpdir)
    res = bass_utils.run_bass_kernel_spmd(
        nc, in_maps, core_ids=list(range(N_CORES)), **kw)
    parts = [res.results[c]["out"] for c in range(N_CORES)]
    out = np.stack([parts[2 * b] + parts[2 * b + 1] for b in range(B)])
    return out, res


def kernel(x, q_proj, k_proj, v_proj, o_proj, token_positions):
    out, _ = run(dict(x=x, q_proj=q_proj, k_proj=k_proj, v_proj=v_proj,
                      o_proj=o_proj, token_positions=token_positions))
    return out
